# revision 32
# baseline (speedup 1.0000x reference)
"""Causal depthwise conv1d (K=4) + SiLU, sharded over 8 NeuronCores.

Full shapes: x [4, 8192, 2048] f32, weight [2048, 4] f32 -> y [4, 8192, 2048] f32.

Strategy: tensor-parallel over the hidden/channel dim (fully channel
independent, no halo exchange). Each core gets 256 channels -> 1024
independent rows (batch x channel). All HBM traffic is bf16 (the 2e-2
rel-err budget dwarfs bf16's ~1e-3), halving the memory-bound roofline
vs f32.

Layout: time is phase-split host-side, t = 4j + p. SBUF partition dim
packs (32 rows x 4 phases); the free dim is the block index j. A causal
conv tap then only ever reads the current block j or block j-1, so each
512-block PSUM chunk needs just TWO 128x128 banded-matmul accumulations
(prev-block taps + cur-block taps) instead of one diag matmul per tap:
2x less TensorEngine time than the diagonal formulation, keeping PE
(~55us) under the bf16 DMA roofline (~94us). The banded weight matrices
(block-diagonal over rows, 4x4 tap bands over phases) are built host-side
and DMA'd once. A leading zero block column provides causal padding.

Compute: PE accumulates bf16 matmuls into f32 PSUM; ACT applies SiLU
straight out of PSUM, writing bf16. Output DMAs are triggered by the
otherwise-idle DVE so the ~900ns semaphore-propagation wait between an
activation and its store never blocks the next activation; inputs stream
on SP's ring. x is stored unpadded (4096B-aligned partition lines — the
odd 2-byte halo column measurably degraded input DMA bandwidth); the
causal zero block is handled by skipping the prev-block matmul's first
output column in chunk 0.

Raw bass (no Tile framework): the installed walrus codegen only accepts one
sync wait per compute instruction, so all synchronization is explicit wait_ge
sequencer instructions. Per-buffer-slot DMA semaphores keep concurrent DMA
completion increments unambiguous. Sem increments fire at instruction
completion, but the sequencer runs ahead, so consumers of an engine's result
always gate on that completion increment (including same-engine self-waits
before DMA triggers).
"""

import contextlib

import numpy as np
import ml_dtypes

B, S, H, K = 4, 8192, 2048, 4
N_CORES = 8
HC = H // N_CORES          # 256 channels per core
ROWS = B * HC              # 1024 rows per core, row r = b*HC + c
P = 4                      # time phases per partition group, t = P*j + p
J = S // P                 # 2048 blocks
RPU = 128 // P             # 32 rows per partition unit
NU = ROWS // RPU           # 32 units (tiles); tile k = unit k, all blocks
NG = HC // RPU             # 8 distinct weight groups (weights repeat per b)
NB = 8                     # buffers per tile kind
NC_CHUNK = 512             # one PSUM bank of fp32
NCHUNKS = J // NC_CHUNK    # 4
PC = 1024                  # psum buffer / activation chunk (2 banks)
NH = J // PC               # 2 chunks per tile
NPS = 4                    # psum buffers (all 8 banks); ping depth 4

BF16 = ml_dtypes.bfloat16

_last_results = None       # test harness introspection (exec_time_ns etc.)
_ACT_FUNC = "Silu"         # sim override hook (CoreSim lacks Silu)


def _build_program():
    from concourse import bass, mybir

    f32 = mybir.dt.float32
    bf16 = mybir.dt.bfloat16
    AF = mybir.ActivationFunctionType

    nc = bass.Bass()
    # phase-split x: row 128*u + 4*rho + p holds x[32u+rho, P*j+p] at col j
    x_d = nc.declare_dram_parameter("x", [NU * 128, J], bf16, isOutput=False)
    # compact per-diagonal scalars (cur: NG*K cols, prev: NG*(K-1) cols,
    # last col zeros for the Silu bias); the dense banded stationaries are
    # assembled on-device by the otherwise-idle GpSimd+DVE to keep 512KB of
    # weight-table DMA off the HBM-saturated stream
    SCC = NG * K + NG * (K - 1) + 1
    sc_d = nc.declare_dram_parameter("sc", [128, SCC], f32, isOutput=False)
    y_d = nc.declare_dram_parameter("y", [NU * 128, J], bf16, isOutput=True)

    with contextlib.ExitStack() as st:
        wsb = st.enter_context(nc.sbuf_tensor("wsb", [128, NG * 2 * 128], bf16))
        scsb = st.enter_context(nc.sbuf_tensor("scsb", [128, SCC], f32))
        ones = st.enter_context(nc.sbuf_tensor("ones", [128, 128], bf16))
        msk = st.enter_context(nc.sbuf_tensor("msk", [128, 7 * 128], bf16))
        tmp = st.enter_context(nc.sbuf_tensor("tmp", [128, 128], bf16))
        xts = [
            st.enter_context(nc.sbuf_tensor(f"xt{i}", [128, J], bf16))
            for i in range(NB)
        ]
        yts = [
            st.enter_context(nc.sbuf_tensor(f"yt{i}", [128, J], bf16))
            for i in range(NB)
        ]
        pss = [
            st.enter_context(nc.psum_tensor(f"ps{i}", [128, PC], f32))
            for i in range(NPS)
        ]

        def wmat(g, which):               # which: 0=prev-block, 1=cur-block
            c0 = (g * 2 + which) * 128
            return wsb[:, c0 : c0 + 128]

        def mdiag(d):                     # shifted-diag mask, delta = d - 3
            return msk[:, d * 128 : (d + 1) * 128]

        with (
            nc.Block() as block,
            nc.semaphore("wsem") as wsem,
            nc.semaphore("act") as act,
            nc.semaphore("pe") as pe,
            nc.semaphore("dl") as dl,
            nc.semaphore("esem") as esem,
            nc.semaphore("dve") as dve,
            nc.semaphore("tl") as tl,
            contextlib.ExitStack() as sems,
        ):
            din = [
                sems.enter_context(nc.semaphore(f"din{i}")) for i in range(NB)
            ]
            dout = [
                sems.enter_context(nc.semaphore(f"dout{i}")) for i in range(NB)
            ]

            @block.gpsimd
            def _(gpsimd):
                # shifted-diagonal masks for the on-device weight build
                # (affine_select only exists on gpsimd). Cur-block masks
                # (d=3..6) first so DVE can start W_cur_0 after 4 incs.
                gpsimd.memset(ones[:, :], 1.0)
                for d in (3, 4, 5, 6, 0, 1, 2):
                    delta = d - 3
                    gpsimd.affine_select(
                        out=mdiag(d), in_=ones[:, :],
                        pattern=[[1, 128]], base=-delta, channel_multiplier=-1,
                        compare_op=mybir.AluOpType.is_equal, fill=0.0,
                    ).then_inc(esem)

            @block.vector
            def _(vector):
                # assemble the 16 banded stationaries: each is a sum of
                # masked shifted diagonals scaled by a per-partition column
                vector.wait_ge(wsem, 16)
                vector.wait_ge(esem, 4)

                def build_wc(g):
                    wc = wmat(g, 1)
                    # cur-block: delta = 0..3, tap K-1-delta
                    vector.tensor_scalar_mul(
                        wc, mdiag(3), scsb[:, g * K : g * K + 1]
                    )
                    for delta in range(1, K):
                        vector.tensor_scalar_mul(
                            tmp[:, :], mdiag(3 + delta),
                            scsb[:, g * K + delta : g * K + delta + 1],
                        )
                        vector.tensor_add(wc, wc, tmp[:, :])

                def build_wp(g):
                    wp = wmat(g, 0)
                    # prev-block: delta = -1..-3, tap -delta-1
                    c0 = NG * K + g * (K - 1)
                    vector.tensor_scalar_mul(
                        wp, mdiag(2), scsb[:, c0 : c0 + 1]
                    )
                    for dp in range(2, K):
                        vector.tensor_scalar_mul(
                            tmp[:, :], mdiag(3 - dp),
                            scsb[:, c0 + dp - 1 : c0 + dp],
                        )
                        mm = vector.tensor_add(wp, wp, tmp[:, :])
                    return mm

                build_wc(0)
                vector.wait_ge(esem, 7)
                build_wp(0).then_inc(dve)
                for g in range(1, NG):
                    build_wc(g)
                    build_wp(g).then_inc(dve)

            @block.sync
            def _(sync):
                sync.dma_start(out=scsb[:, :], in_=sc_d[:, :]).then_inc(wsem, 16)
                for k in range(NU):
                    if k >= NB:
                        # xt slot free once PE consumed tile k-NB
                        sync.wait_ge(pe, NH * (k - NB + 1))
                    if k == NU - 1:
                        # split the last tile's load per PC-chunk so the tail
                        # PE/ACT/store pipeline starts before the full tile
                        # lands. The halves complete out of order across the
                        # DMA engines, so half B gets its own semaphore —
                        # mixing both halves' incs on din would let PE start
                        # chunk 0 before half A fully landed.
                        for h, sem in ((0, din[k % NB]), (1, dl)):
                            sync.dma_start(
                                out=xts[k % NB][:, h * PC : (h + 1) * PC],
                                in_=x_d[k * 128 : (k + 1) * 128, h * PC : (h + 1) * PC],
                            ).then_inc(sem, 16)
                    else:
                        sync.dma_start(
                            out=xts[k % NB][:, :],
                            in_=x_d[k * 128 : (k + 1) * 128, :],
                        ).then_inc(din[k % NB], 16)

            @block.tensor
            def _(tensor):
                # dummy matmuls on garbage SBUF (no waits): keep PE busy
                # through its cold p-state window so the first real chunks
                # run at full clock. pss[0] is reset by chunk 0's start=True.
                for _ in range(6):
                    tensor.matmul(
                        pss[0][:, 0:NC_CHUNK],
                        ones[:, :],
                        xts[0][:, 0:NC_CHUNK],
                        start=True,
                        stop=True,
                        skip_group_check=True,
                    )
                # pe/act semaphores count PC-col chunks, NH per tile; psum
                # buffers rotate over NPS chunks
                for k in range(NU):
                    if k < NG:
                        # stationary pair g=k built by DVE
                        tensor.wait_ge(dve, k + 1)
                    if k < NU - 1:
                        tensor.wait_ge(din[k % NB], 16 * (k // NB + 1))
                    xt = xts[k % NB]
                    g = k % NG
                    for h in range(NH):
                        G = k * NH + h
                        if k == NU - 1:
                            # split load: chunk 0 needs half A, chunk 1 both
                            if h == 0:
                                tensor.wait_ge(din[k % NB], 16 * (k // NB + 1))
                            else:
                                tensor.wait_ge(dl, 16)
                        if G >= NPS:
                            # psum buffer free once silu of chunk G-NPS done
                            tensor.wait_ge(act, G - NPS + 1)
                        ps = pss[G % NPS]
                        for c2 in range(PC // NC_CHUNK):
                            c0 = h * PC + c2 * NC_CHUNK   # within the tile
                            p0 = c2 * NC_CHUNK            # within the psum buf
                            if c0 == 0:
                                # block -1 is the causal zero block: psum col
                                # 0 gets no prev contribution. cur starts the
                                # group (zeroes the whole 512-col bank).
                                mm = tensor.matmul(
                                    ps[:, 0:NC_CHUNK],
                                    wmat(g, 1),
                                    xt[:, 0:NC_CHUNK],
                                    start=True,
                                    stop=False,
                                    skip_group_check=True,
                                )
                                mm = tensor.matmul(
                                    ps[:, 1:NC_CHUNK],
                                    wmat(g, 0),
                                    xt[:, 0 : NC_CHUNK - 1],
                                    start=False,
                                    stop=True,
                                    skip_group_check=True,
                                )
                            else:
                                mm = tensor.matmul(
                                    ps[:, p0 : p0 + NC_CHUNK],
                                    wmat(g, 0),
                                    xt[:, c0 - 1 : c0 - 1 + NC_CHUNK],
                                    start=True,
                                    stop=False,
                                    skip_group_check=True,
                                )
                                mm = tensor.matmul(
                                    ps[:, p0 : p0 + NC_CHUNK],
                                    wmat(g, 1),
                                    xt[:, c0 : c0 + NC_CHUNK],
                                    start=False,
                                    stop=True,
                                    skip_group_check=True,
                                )
                            if G == NU * NH - 1 and c2 == 0:
                                # tail: first psum bank of the final chunk
                                # ready — lets its half-activation start
                                mm.then_inc(tl)
                        mm.then_inc(pe)

            @block.scalar
            def _(scalar):
                func = getattr(AF, _ACT_FUNC)

                def store_chunk(G):
                    # runs while the next activation occupies the engine, so
                    # chunk G's completion inc has already propagated: the
                    # wait is ~free and the ~900ns semaphore latency stays
                    # off the activation chain. Chunk-granular stores keep
                    # the output queue only ~1 chunk behind the data.
                    k, h = G // NH, G % NH
                    scalar.wait_ge(act, G + 1)
                    scalar.dma_start(
                        out=y_d[k * 128 : (k + 1) * 128, h * PC : (h + 1) * PC],
                        in_=yts[k % NB][:, h * PC : (h + 1) * PC],
                    ).then_inc(dout[k % NB], 16)

                GL = NU * NH - 1               # final chunk, handled below
                for k in range(NU):
                    for h in range(NH):
                        G = k * NH + h
                        if G == GL:
                            continue
                        scalar.wait_ge(pe, G + 1)
                        if h == 0 and k >= NB:
                            # yt slot's previous stores (tile k-NB) done;
                            # total-count gate, so the two chunks' incs
                            # mixing on one semaphore is unambiguous
                            scalar.wait_ge(dout[k % NB], 16 * NH * (k // NB))
                        scalar.activation(
                            out=yts[k % NB][:, h * PC : (h + 1) * PC],
                            in_=pss[G % NPS][:, :],
                            func=func,
                            bias=0.0 if func == AF.Copy else scsb[:, SCC - 1 : SCC],
                            scale=1.0,
                        ).then_inc(act)
                        if G >= 1:
                            store_chunk(G - 1)
                # tail: the final chunk runs as two 512-col halves so its
                # last store launches half an activation earlier
                slot = (NU - 1) % NB
                r0 = (NU - 1) * 128
                y0 = (NH - 1) * PC
                pl = pss[GL % NPS]
                scalar.wait_ge(tl, 1)
                scalar.activation(
                    out=yts[slot][:, y0 : y0 + NC_CHUNK],
                    in_=pl[:, 0:NC_CHUNK],
                    func=func,
                    bias=0.0 if func == AF.Copy else scsb[:, SCC - 1 : SCC],
                    scale=1.0,
                ).then_inc(dl)
                store_chunk(GL - 1)
                scalar.wait_ge(pe, GL + 1)
                scalar.activation(
                    out=yts[slot][:, y0 + NC_CHUNK : y0 + PC],
                    in_=pl[:, NC_CHUNK:PC],
                    func=func,
                    bias=0.0 if func == AF.Copy else scsb[:, SCC - 1 : SCC],
                    scale=1.0,
                ).then_inc(act)
                scalar.wait_ge(dl, 17)
                scalar.dma_start(
                    out=y_d[r0 : r0 + 128, y0 : y0 + NC_CHUNK],
                    in_=yts[slot][:, y0 : y0 + NC_CHUNK],
                ).then_inc(dout[slot], 16)
                scalar.wait_ge(act, GL + 1)
                scalar.dma_start(
                    out=y_d[r0 : r0 + 128, y0 + NC_CHUNK : y0 + PC],
                    in_=yts[slot][:, y0 + NC_CHUNK : y0 + PC],
                ).then_inc(dout[slot], 16)
                for i in range(NB):
                    n_tiles = len([k for k in range(NU) if k % NB == i])
                    extra = 16 if i == slot else 0
                    scalar.wait_ge(dout[i], 16 * NH * n_tiles + extra)

    return nc


def _scalar_table(ws):
    """ws: (HC, K) f32 -> [128, NG*(2K-1)+1] f32 per-diagonal scalar columns.

    Partition index q = 4*rho + p_in; the device scatters column s onto the
    shifted diagonal [q, q+delta]. Cur-block (delta = p_out - p_in in 0..K-1)
    carries tap i = K-1-delta, valid while (q%P)+delta <= P-1; prev-block
    (delta = -1..-(K-1)) carries tap i = -delta-1, valid while (q%P) >= -delta.
    The final column is zeros (Silu bias operand).
    """
    q = np.arange(128)
    sc = np.zeros((128, NG * (2 * K - 1) + 1), np.float32)
    for g in range(NG):
        ch = ws[RPU * g : RPU * (g + 1)]          # (RPU, K)
        wq = ch[q // P, :]                        # (128, K) per-partition taps
        for delta in range(K):
            sc[:, g * K + delta] = wq[:, K - 1 - delta] * ((q % P) + delta <= P - 1)
        for dp in range(1, K):
            sc[:, NG * K + g * (K - 1) + dp - 1] = wq[:, dp - 1] * ((q % P) >= dp)
    return sc


def kernel(x, weight):
    global _last_results
    from concourse.bass_utils import run_bass_kernel_spmd

    x = np.asarray(x, dtype=np.float32)
    weight = np.asarray(weight, dtype=np.float32)

    nc = _build_program()

    in_maps = []
    for core in range(N_CORES):
        sl = slice(core * HC, (core + 1) * HC)
        # [B, S, HC] -> [B, HC, S] -> [ROWS, S], row r = b*HC + c
        xc = x[:, :, sl].transpose(0, 2, 1).reshape(ROWS, S)
        # phase split: row 4r+p, col j = x[r, 4j+p]
        xs = np.ascontiguousarray(
            xc.reshape(ROWS, J, P).transpose(0, 2, 1).reshape(ROWS * P, J)
        ).astype(BF16)
        in_maps.append({"x": xs, "sc": _scalar_table(weight[sl, :])})

    res = run_bass_kernel_spmd(nc, in_maps, list(range(N_CORES)))
    _last_results = res

    out = np.empty((B, S, H), np.float32)
    for core in range(N_CORES):
        sl = slice(core * HC, (core + 1) * HC)
        yc = np.asarray(res.results[core]["y"], dtype=np.float32)
        # undo phase split, then row-major [B, HC, S] -> [B, S, HC]
        yc = yc.reshape(ROWS, P, J).transpose(0, 2, 1).reshape(B, HC, S)
        out[:, :, sl] = yc.transpose(0, 2, 1)
    return out
